# revision 38
# baseline (speedup 1.0000x reference)
"""Trainium2 Bass kernel: attention with rotary embedding + XL memory.

Model (B=2, T=1024, D=2048, H=16, hd=128, XL=1024):
  qkv = x @ w_qkv.T ; split q,k,v ; k_xl += pos_emb ; rope(q), rope(k)
  per head: scores = q @ [k_xl | k].T / sqrt(hd) ; softmax ; y = P @ [v_xl | v]
  out = y @ w_proj.T

Sharding: 8 cores = 2 batches x 4 head-groups (4 heads each). Each core
computes its head-group's qkv projection, rope, attention, and a partial
output projection (contraction over its 512 y-channels); the host sums the
4 partials per batch (tensor-parallel unshard) and concatenates batches.

Device design notes:
  - All matmul inputs are float32r (FP22 read) for full PE rate; tiles and
    DRAM tensors feeding matmuls are declared f32r so every producer
    (DMA or compute op) satisfies the walrus rounded-producer check.
  - GEMMs run on transposed operands (host-side layout prep) so the PE
    contraction dim is always the partition dim.
  - RoPE pair (2i, 2i+1) becomes block pair (i, 64+i) via a host-side
    permutation of W_q / W_k rows (and k_xl / pos_emb columns). The
    cross-half combine uses ACT rebase copies (tensor_tensor requires
    same start partitions; plain copies do not).
  - Scores are computed transposed (S.T = [kt, qt]) so softmax'd P.T feeds
    the AV matmul directly as rhs. Softmax skips max-subtraction (scores
    are ~N(0,1); exp is fp32-safe). The denominator is accumulated with DVE
    adds + a ones-matmul partition reduce; normalization is applied to y.
  - Since each engine executes in program order, emission is interleaved
    for overlap: all 4 heads' attention chunks round-robin (ACT exp latency
    hides under other heads' matmuls), AV matmuls trail scores by one chunk,
    and the tb1 V-GEMM / tb0 output projection are woven into attention
    chunk slots as PE gap fillers.
"""
import sys

sys.path.insert(0, "/opt/trn_rl_repo")

import numpy as np

import concourse.bass as bass  # noqa: F401
import concourse.mybir as mybir
import concourse.tile as tile
from concourse import bacc
from concourse.bass import ts
from concourse.bass_utils import run_bass_kernel_spmd  # noqa: F401 (fallback)

F32 = mybir.dt.float32
F32R = mybir.dt.float32r
AF = mybir.ActivationFunctionType
ADD = mybir.AluOpType.add

B, T, D = 2, 1024, 2048
H, HD, XL = 16, 128, 1024
HPC = 4                 # heads per core
CPB = 4                 # cores per batch
NCORES = 8
NCC = D // 128          # 16 contraction chunks
SCALE = 1.0 / np.sqrt(HD)

_CACHE: dict = {}


def _build_nc():
    nc = bacc.Bacc("TRN2", target_bir_lowering=False, debug=False)

    x_d = nc.dram_tensor("x", [2, 128, NCC, 512], F32R, kind="ExternalInput")
    wqk_d = nc.dram_tensor("wqk", [8, 128, NCC, 128], F32R, kind="ExternalInput")
    wv_d = nc.dram_tensor("wv", [2, 128, NCC, 256], F32R, kind="ExternalInput")
    cs_d = nc.dram_tensor("cs", [2, 128, T], F32, kind="ExternalInput")
    kxl_d = nc.dram_tensor("kxl", [128, 4, XL], F32R, kind="ExternalInput")
    pos_d = nc.dram_tensor("pos", [128, 4, XL], F32R, kind="ExternalInput")
    vxl_d = nc.dram_tensor("vxl", [128, 8, 512], F32R, kind="ExternalInput")
    wproj_d = nc.dram_tensor("wproj", [16, 128, 4, 128], F32R, kind="ExternalInput")
    out_d = nc.dram_tensor("out", [16, 2, 128, 512], F32, kind="ExternalOutput")

    with tile.TileContext(nc) as tc, nc.allow_low_precision(
            reason="fp32r matmul inputs: FP22 rounding is intended"):
        with (
            tc.tile_pool(name="const", bufs=1) as const,
            tc.tile_pool(name="xp", bufs=1) as xp,
            tc.tile_pool(name="wqkp", bufs=2) as wqkp,
            tc.tile_pool(name="wvp", bufs=1) as wvp,
            tc.tile_pool(name="wpp", bufs=4) as wpp,
            tc.tile_pool(name="ptp", bufs=5) as ptp,
            tc.tile_pool(name="ropep", bufs=1) as ropep,
            tc.tile_pool(name="accp", bufs=4) as accp,
            tc.tile_pool(name="smallp", bufs=1) as smallp,
            tc.tile_pool(name="outp", bufs=2) as outp,
            tc.tile_pool(name="psum", bufs=4, space="PSUM") as psum,
            tc.tile_pool(name="pyp", bufs=4, space="PSUM") as pyp,
        ):
            # ---- persistent tiles ----
            cc = const.tile([128, T], F32, tag="cc")   # [cos; cos]
            ss = const.tile([128, T], F32, tag="ss")   # [-sin; +sin]
            ones = const.tile([128, 128], F32R, tag="ones")
            qk = const.tile([128, 8, T], F32R, tag="qk")   # roped qT (0-3), kT (4-7)
            vsb = const.tile([128, 8, 512], F32R, tag="vsb")  # v, [t, d] natural
            ysb = const.tile([128, 4, T], F32R, tag="ysb")    # y.T per head

            ones_f = outp.tile([128, 128], F32, tag="ot")
            nc.vector.memset(ones_f[:], 1.0)
            nc.vector.tensor_copy(ones[:], ones_f[:])

            # ---- phase 1: QKV projection (+rope) per query t-block ----
            for tb in range(2):
                tbsl = ts(tb, 512)
                pre_wt = None
                if tb == 0:
                    pre_wt = wqkp.tile([128, NCC, 128], F32R, tag="wqk",
                                       name="wt_pre")
                    for j4 in range(4):
                        nc.sync.dma_start(pre_wt[:, 4 * j4:4 * j4 + 4, :],
                                          wqk_d[0, :, 4 * j4:4 * j4 + 4, :])
                xt = xp.tile([128, NCC, 512], F32R, tag="x")
                for j in range(8):
                    nc.sync.dma_start(xt[:, 2 * j:2 * j + 2, :],
                                      x_d[tb, :, 2 * j:2 * j + 2, :])
                if tb == 0:
                    nc.sync.dma_start(cc[:], cs_d[0])
                    nc.sync.dma_start(ss[:], cs_d[1])
                # q/k in transposed layout [d, t], fused rope out of PSUM
                for f in range(8):
                    if tb == 0 and f == 0:
                        wt = pre_wt
                    else:
                        wt = wqkp.tile([128, NCC, 128], F32R, tag="wqk")
                        for j4 in range(4):
                            nc.sync.dma_start(wt[:, 4 * j4:4 * j4 + 4, :],
                                              wqk_d[f, :, 4 * j4:4 * j4 + 4, :])
                    pmm = psum.tile([128, 512], F32, tag="ps")
                    for ci in range(NCC):
                        nc.tensor.matmul(pmm[:], wt[:, ci, :], xt[:, ci, :],
                                         start=(ci == 0), stop=(ci == NCC - 1))
                    # packed rope: new = P*[cos;cos] + swap(P)*[-sin;+sin]
                    # (swap via ACT rebase copies; TT ops need same bases)
                    sw = ropep.tile([128, 512], F32, tag="sw")
                    nc.scalar.copy(sw[0:64, :], pmm[64:128, :])
                    nc.scalar.copy(sw[64:128, :], pmm[0:64, :])
                    dst = qk[:, f, tbsl]
                    t2 = ropep.tile([128, 512], F32, tag="t2")
                    nc.vector.tensor_mul(dst, pmm[:], cc[:, tbsl])
                    nc.vector.tensor_mul(t2[:], sw[:], ss[:, tbsl])
                    nc.vector.tensor_add(dst, dst, t2[:])
                # v in natural layout [t, d]; tb1's v is deferred into the
                # attention-tb0 chunk slots (PE gap filler)
                def emit_v_group(tb, half, tt, wvt, xt=xt):
                    pv = psum.tile([128, 256], F32, tag="ps", name="pv")
                    for ci in range(NCC):
                        nc.tensor.matmul(pv[:], xt[:, ci, ts(tt, 128)],
                                         wvt[:, ci, :],
                                         start=(ci == 0), stop=(ci == NCC - 1))
                    nc.scalar.copy(vsb[:, tb * 4 + tt, ts(half, 256)], pv[:])

                if tb == 0:
                    for half in range(2):
                        wvt = wvp.tile([128, NCC, 256], F32R, tag="wv")
                        for j in range(8):
                            nc.sync.dma_start(wvt[:, 2 * j:2 * j + 2, :],
                                              wv_d[half, :, 2 * j:2 * j + 2, :])
                        for tt in range(4):
                            emit_v_group(0, half, tt, wvt)
                else:
                    v_fillers = []
                    for half in range(2):
                        def load_wv(half=half):
                            wvt = wvp.tile([128, NCC, 256], F32R, tag="wv",
                                           name=f"wvt1_{half}")
                            for j in range(8):
                                nc.sync.dma_start(
                                    wvt[:, 2 * j:2 * j + 2, :],
                                    wv_d[half, :, 2 * j:2 * j + 2, :])
                            return wvt
                        for tt in range(4):
                            def filler(half=half, tt=tt, load_wv=load_wv,
                                       xt=xt):
                                if tt == 0:
                                    filler.wvt = load_wv()
                                emit_v_group(1, half, tt, filler.wvt, xt)
                            v_fillers.append(filler)

            # ---- XL memory: load once (kxl gets pos added via accum-DMA) ----
            kxl = const.tile([128, 4, XL], F32R, tag="kxl")
            vxl = const.tile([128, 8, 512], F32R, tag="vxl")
            for j in range(4):
                nc.sync.dma_start(kxl[:, j, 0:512], kxl_d[:, j, 0:512])
                nc.sync.dma_start(kxl[:, j, 512:1024], kxl_d[:, j, 512:1024])
            for j in range(4):
                nc.gpsimd.dma_start(kxl[:, j, :], pos_d[:, j, :], accum_op=ADD)
            for j in range(8):
                nc.sync.dma_start(vxl[:, j, :], vxl_d[:, j, :])

            # ---- phase 2: attention + projection, interleaved ----
            def emit_proj(ob, tb):
                wpt = wpp.tile([128, 4, 128], F32R, tag="wp")
                nc.sync.dma_start(wpt[:, 0:2, :], wproj_d[ob, :, 0:2, :])
                nc.sync.dma_start(wpt[:, 2:4, :], wproj_d[ob, :, 2:4, :])
                po = psum.tile([128, 512], F32, tag="ps")
                for yc in range(4):
                    nc.tensor.matmul(po[:], wpt[:, yc, :],
                                     ysb[:, yc, ts(tb, 512)],
                                     start=(yc == 0), stop=(yc == 3))
                ot = outp.tile([128, 512], F32, tag="ot")
                nc.vector.tensor_copy(ot[:], po[:])
                nc.sync.dma_start(out_d[ob, tb], ot[:])

            def attn_quad(tb, fillers, every=2):
                """Chunk-interleaved attention for all 4 heads; `fillers`
                are callables emitted inside chunk slots (PE gap fillers:
                deferred v-GEMM groups or projection blocks)."""
                tbsl = ts(tb, 512)
                py, acc = {}, {}
                for h in range(4):
                    py[h] = pyp.tile([128, 512], F32, tag="py", name=f"py{h}")
                    acc[h] = accp.tile([128, 512], F32R, tag="acc",
                                       name=f"acc{h}")
                fill = list(fillers)
                pend = {}      # (h -> (pt, lv, kc)) av deferred by one chunk
                def emit_av(h):
                    pt_, lv_, kc_ = pend.pop(h)
                    nc.tensor.matmul(py[h][:], lv_, pt_[:],
                                     start=(kc_ == 0), stop=(kc_ == 15))
                for kc in range(16):
                    for h in range(4):
                        if kc < 8:
                            lk = kxl[:, h, ts(kc, 128)]
                            lv = vxl[:, kc, ts(h, 128)]
                        else:
                            lk = qk[:, 4 + h, ts(kc - 8, 128)]
                            lv = vsb[:, kc - 8, ts(h, 128)]
                        pss = psum.tile([128, 512], F32, tag="ps")
                        nc.tensor.matmul(pss[:], lk, qk[:, h, tbsl],
                                         start=True, stop=True)
                        pt = ptp.tile([128, 512], F32R, tag="pt")
                        nc.scalar.activation(pt[:], pss[:], AF.Exp, scale=SCALE)
                        if kc == 0:
                            nc.vector.tensor_copy(acc[h][:], pt[:])
                        else:
                            nc.vector.tensor_add(acc[h][:], acc[h][:], pt[:])
                        if h in pend:
                            emit_av(h)
                        pend[h] = (pt, lv, kc)
                    if kc % every == every - 1 and fill:
                        fill.pop(0)()
                for h in range(4):
                    emit_av(h)
                for h in range(4):
                    pden_t = psum.tile([128, 512], F32, tag="ps")
                    pden = pden_t[0:1, :]
                    nc.tensor.matmul(pden, ones[:, 0:1], acc[h][:],
                                     start=True, stop=True)
                    rec = smallp.tile([1, 512], F32R, tag="rec")
                    nc.vector.reciprocal(rec[:], pden)
                    pbc = psum.tile([128, 512], F32, tag="ps")
                    nc.tensor.matmul(pbc[:], ones[0:1, :], rec[:],
                                     start=True, stop=True)
                    rbc = smallp.tile([128, 512], F32, tag="rbc")
                    nc.scalar.copy(rbc[:], pbc[:])
                    nc.vector.tensor_mul(ysb[:, h, tbsl], py[h][:], rbc[:])
                while fill:
                    fill.pop(0)()

            # v-tb1 group g must precede the first av that reads vsb chunk
            # g (kc=8+g, emitted at iteration 9+g due to the av stagger);
            # slot 2g+1 <= 9+g holds for g <= 7.
            attn_quad(0, v_fillers, every=2)
            attn_quad(1, [lambda ob=ob: emit_proj(ob, 0) for ob in range(16)],
                      every=1)
            for ob in range(16):                  # proj tb1
                emit_proj(ob, 1)


    nc.compile()
    return nc


def _get_nc():
    if "nc" not in _CACHE:
        _CACHE["nc"] = _build_nc()
    return _CACHE["nc"]


_PERM = np.concatenate([np.arange(0, HD, 2), np.arange(1, HD, 2)])
_PP = np.concatenate([_PERM + i * HD for i in range(HPC)])  # per-head-block perm


def make_in_maps(x, cos, sin, k_xl, v_xl, pos_emb, w_qkv, w_proj):
    """Host-side shard + layout prep: one input dict per core."""
    x = np.asarray(x, np.float32)
    cos = np.asarray(cos, np.float32)
    sin = np.asarray(sin, np.float32)
    k_xl = np.asarray(k_xl, np.float32)
    v_xl = np.asarray(v_xl, np.float32)
    pos_emb = np.asarray(pos_emb, np.float32)
    w_qkv = np.asarray(w_qkv, np.float32)
    w_proj = np.asarray(w_proj, np.float32)

    # cs[0] = [cos; cos] ; cs[1] = [-sin; +sin]  (packed-rope factors)
    cs = np.ascontiguousarray(np.stack([
        np.concatenate([cos.T, cos.T], axis=0),
        np.concatenate([-sin.T, sin.T], axis=0),
    ]))

    in_maps = []
    for c in range(NCORES):
        b, g = c // CPB, c % CPB
        h0 = g * HPC
        cols = slice(h0 * HD, (h0 + HPC) * HD)

        # x: [tb, pi, po, tl]
        x_arr = np.ascontiguousarray(
            x[b].T.reshape(NCC, 128, 2, 512).transpose(2, 1, 0, 3))
        # w_q/w_k rows for this head group, rope-permuted; [f, pi, ci, fcol]
        wq = w_qkv[0 * D + h0 * HD:0 * D + (h0 + HPC) * HD][_PP]
        wk = w_qkv[1 * D + h0 * HD:1 * D + (h0 + HPC) * HD][_PP]
        wqk_rows = np.concatenate([wq, wk], axis=0)  # [1024, D]
        wqk_arr = np.ascontiguousarray(
            wqk_rows.reshape(8, 128, NCC, 128).transpose(0, 3, 2, 1))
        # w_v rows (unpermuted); [half, pi, ci, col]
        wv_rows = w_qkv[2 * D + h0 * HD:2 * D + (h0 + HPC) * HD]  # [512, D]
        wv_arr = np.ascontiguousarray(
            wv_rows.reshape(2, 256, NCC, 128).transpose(0, 3, 2, 1))
        # k_xl / pos_emb: permuted cols, transposed; [pi, j, t]
        kxlT = k_xl[b][:, cols][:, _PP].T  # [512, XL]
        kxl_arr = np.ascontiguousarray(
            kxlT.reshape(4, 128, XL).transpose(1, 0, 2))
        posT = pos_emb[:, cols][:, _PP].T
        pos_arr = np.ascontiguousarray(
            posT.reshape(4, 128, XL).transpose(1, 0, 2))
        # v_xl natural; [pi, j, col]
        vxl_arr = np.ascontiguousarray(
            v_xl[b][:, cols].reshape(8, 128, 512).transpose(1, 0, 2))
        # w_proj column block, transposed; [ob, pi, yc, ocol]
        wprojT = w_proj[:, cols].T  # [512, D]
        wproj_arr = np.ascontiguousarray(
            wprojT.reshape(4, 128, 16, 128).transpose(2, 1, 0, 3))

        in_maps.append({
            "x": x_arr, "wqk": wqk_arr, "wv": wv_arr, "cs": cs,
            "kxl": kxl_arr, "pos": pos_arr, "vxl": vxl_arr,
            "wproj": wproj_arr,
        })
    return in_maps


def unshard(results):
    """results: list of 8 dicts with 'out' [16, 2, 128, 512] -> [B, T, D]."""
    out = np.zeros((B, T, D), np.float32)
    for c in range(NCORES):
        b = c // CPB
        outT = np.asarray(results[c]["out"]).transpose(0, 2, 1, 3).reshape(D, T)
        out[b] += outT.T
    return out


def _get_runner():
    """Persistent jitted 8-core executable (avoids per-call retrace of the
    bass2jax lowering; the NEFF itself is cached by neuronx-cc)."""
    if "runner" in _CACHE:
        return _CACHE["runner"]
    import jax
    import jax.numpy as jnp
    from jax.sharding import Mesh, PartitionSpec, NamedSharding
    from jax.experimental.shard_map import shard_map
    from concourse.bass2jax import (_bass_exec_p, partition_id_tensor,
                                    install_neuronx_cc_hook)

    nc = _get_nc()
    install_neuronx_cc_hook()
    in_names, out_names, out_avals, zero_shapes = [], [], [], []
    for alloc in nc.m.functions[0].allocations:
        if not isinstance(alloc, mybir.MemoryLocationSet):
            continue
        name = alloc.memorylocations[0].name
        if alloc.kind == "ExternalInput":
            if nc.partition_id_tensor is None or \
                    name != nc.partition_id_tensor.name:
                in_names.append(name)
        elif alloc.kind == "ExternalOutput":
            shape = tuple(alloc.tensor_shape)
            np_dt = mybir.dt.np(alloc.dtype)
            out_names.append(name)
            out_avals.append(jax.core.ShapedArray(shape, np_dt))
            zero_shapes.append((shape, np_dt))
    n_params, n_outs = len(in_names), len(out_names)
    all_in = in_names + out_names
    if nc.partition_id_tensor is not None:
        all_in = all_in + [nc.partition_id_tensor.name]

    def _body(*args):
        operands = list(args)
        if nc.partition_id_tensor is not None:
            operands.append(partition_id_tensor())
        return tuple(_bass_exec_p.bind(
            *operands, out_avals=tuple(out_avals), in_names=tuple(all_in),
            out_names=tuple(out_names), lowering_input_output_aliases=(),
            sim_require_finite=True, sim_require_nnan=True, nc=nc))

    devices = jax.devices()[:NCORES]
    mesh = Mesh(np.asarray(devices), ("core",))
    fn = jax.jit(
        shard_map(_body, mesh=mesh,
                  in_specs=(PartitionSpec("core"),) * (n_params + n_outs),
                  out_specs=(PartitionSpec("core"),) * n_outs,
                  check_rep=False),
        donate_argnums=tuple(range(n_params, n_params + n_outs)),
        keep_unused=True)
    sharding = NamedSharding(mesh, PartitionSpec("core"))
    zfn = jax.jit(
        lambda: tuple(jnp.zeros((NCORES * s[0], *s[1:]), d)
                      for s, d in zero_shapes),
        out_shardings=(sharding,) * n_outs)
    runner = (fn, zfn, in_names, out_names, out_avals, sharding)
    _CACHE["runner"] = runner
    return runner


def kernel(x, cos, sin, k_xl, v_xl, pos_emb, w_qkv, w_proj, is_causal=0,
           **_ignored):
    # is_causal is 0 for this problem spec (fill=arange, shape []); the
    # non-causal path is the only one implemented.
    import jax
    in_maps = make_in_maps(x, cos, sin, k_xl, v_xl, pos_emb, w_qkv, w_proj)
    fn, zfn, in_names, out_names, out_avals, sharding = _get_runner()
    concat_in = [
        jax.device_put(
            np.concatenate([in_maps[c][nm] for c in range(NCORES)], axis=0),
            sharding)
        for nm in in_names]
    outs = fn(*concat_in, *zfn())
    results = [
        {nm: np.asarray(outs[i]).reshape(NCORES, *out_avals[i].shape)[c]
         for i, nm in enumerate(out_names)}
        for c in range(NCORES)]
    _CACHE["last_results"] = None
    return unshard(results)
